# revision 6
# baseline (speedup 1.0000x reference)
"""Causal single-head attention on 8 Trainium2 NeuronCores.

Shapes (hardcoded per problem spec):
  input_tensor [512, 256, 384] f32, Wq/Wk/Wv [384, 64] f32 -> out [512, 256, 64] f32

Sharding: data-parallel on the batch dim, 64 batches per core, weights
replicated.

Per-batch pipeline on each core (S=256 split into two 128-row blocks,
E=384 split into three 128-row chunks):
  1. DMA x_b [256,384] into SBUF naturally (contiguous rows).
  2. PE-transpose the six 128x128 blocks -> xT [384(3 chunks),256] (PSUM->SBUF).
  3. qkT = [Wq|Wk].T @ xT   -> [128,256] (rows 0:64 = qT, 64:128 = kT).
  4. vT  = Wv.T @ xT        -> [64,256]; PE-transpose back to v [256,64] and
     append a ones column (gives the softmax denominator for free in step 7).
  5. sT[k,q] = kT_block.T @ qT for both k blocks -> [128,2,256] in PSUM.
  6. p = exp(0.125 * sT) on ScalarE (no max subtraction: scores ~ N(0,1), the
     softmax is shift-invariant and exp stays in a safe f32 range), then
     multiply the two diagonal blocks by an upper-triangular 0/1 mask (k<=q).
  7. out_unnorm[q,:] = p_block.T @ [v|1]: col 64 accumulates sum_k p = the
     softmax denominator l; accumulate over the causal k blocks only.
  8. out = out_unnorm[:, :64] * (1/l), DMA to HBM.

Matmuls run as float32r (full PE rate at N>=256); transposes stay plain f32
(exact). Accumulation is always f32 in PSUM.
"""

import numpy as np

import concourse.bass as bass
import concourse.mybir as mybir
import concourse.tile as tile
from concourse import bacc
from concourse.bass import ds, ts
from concourse.bass_utils import run_bass_kernel_spmd
from concourse.masks import make_identity, make_upper_triangular

EMBED = 384
HEAD_DIM = 64
SEQ = 256
BATCH = 512
NCORES = 8
NB = BATCH // NCORES  # batches per core

F32 = mybir.dt.float32
F32R = mybir.dt.float32r

EC = EMBED // 128  # 3 embed chunks
ST = SEQ // 128    # 2 seq blocks


def _build(nb=NB, mm_dt="f32r"):
    """Build the per-core Bass program for nb batches."""
    MD = F32R if mm_dt == "f32r" else F32

    nc = bacc.Bacc("TRN2", target_bir_lowering=False)
    x = nc.dram_tensor("x", [nb, SEQ, EMBED], F32, kind="ExternalInput")
    wq = nc.dram_tensor("wq", [EMBED, HEAD_DIM], MD, kind="ExternalInput")
    wk = nc.dram_tensor("wk", [EMBED, HEAD_DIM], MD, kind="ExternalInput")
    wv = nc.dram_tensor("wv", [EMBED, HEAD_DIM], MD, kind="ExternalInput")
    out = nc.dram_tensor("out", [nb, SEQ, HEAD_DIM], F32, kind="ExternalOutput")

    xv = x[:, :, :].rearrange("b (t p) e -> b p t e", p=128)
    ov = out[:, :, :].rearrange("b (t p) d -> b p t d", p=128)

    with tile.TileContext(nc) as tc:
        with (
            tc.tile_pool(name="const", bufs=1) as cpool,
            tc.tile_pool(name="sb_x", bufs=3) as sb_x,
            tc.tile_pool(name="sb_xt", bufs=3) as sb_xt,
            tc.tile_pool(name="sb_qk", bufs=3) as sb_qk,
            tc.tile_pool(name="sb_v", bufs=3) as sb_v,
            tc.tile_pool(name="sb_p", bufs=3) as sb_p,
            tc.tile_pool(name="sb_o", bufs=3) as sb_o,
            tc.tile_pool(name="ps_xt", bufs=1, space="PSUM") as ps_xt,
            tc.tile_pool(name="ps_qk", bufs=2, space="PSUM") as ps_qk,
            tc.tile_pool(name="ps_vt", bufs=1, space="PSUM") as ps_vt,
            tc.tile_pool(name="ps_vn", bufs=1, space="PSUM") as ps_vn,
            tc.tile_pool(name="ps_st", bufs=1, space="PSUM") as ps_st,
            tc.tile_pool(name="ps_av", bufs=1, space="PSUM") as ps_av,
        ):
            ident = cpool.tile([128, 128], F32)
            make_identity(nc, ident)
            # tri[k, q] = 1.0 where k <= q else 0.0 (f32 staging: memset /
            # affine_select can't write f32r; a DVE copy casts+rounds)
            tri_f32 = cpool.tile([128, 128], F32)
            make_upper_triangular(nc, tri_f32, val=1.0, diag=True)
            if MD is F32:
                tri = tri_f32
            else:
                tri = cpool.tile([128, 128], MD)
                nc.vector.tensor_copy(tri[:, :], tri_f32[:, :])
            ones_f32 = cpool.tile([128, ST, 2], F32)
            nc.vector.memset(ones_f32[:, :, :], 1.0)

            # [Wk|Wv] packed: projection puts kT at partitions 0:64 (base 0,
            # as the scores matmul needs) and vT at 64:128 (only feeds the
            # PE transpose, which works at base 64 with ident[64:,64:]).
            wkv_sb = cpool.tile([128, EC, 128], MD)
            nc.sync.dma_start(
                out=wkv_sb[:, :, 0:HEAD_DIM],
                in_=wk[:, :].rearrange("(c p) d -> p c d", p=128),
            )
            nc.sync.dma_start(
                out=wkv_sb[:, :, HEAD_DIM:128],
                in_=wv[:, :].rearrange("(c p) d -> p c d", p=128),
            )
            wq_sb = cpool.tile([128, EC, HEAD_DIM], MD)
            nc.sync.dma_start(
                out=wq_sb[:, :, :],
                in_=wq[:, :].rearrange("(c p) d -> p c d", p=128),
            )

            for b in range(nb):
                # 1. load x_b
                xs = sb_x.tile([128, ST, EMBED], F32, tag="xs")
                nc.sync.dma_start(out=xs[:, :, :], in_=xv[b])

                # 2. transpose x -> xT  (blocks at col c*256 + t*128)
                xt_ps = ps_xt.tile([128, EC * SEQ], F32, tag="xt")
                for t in range(ST):
                    for c in range(EC):
                        nc.tensor.transpose(
                            xt_ps[:, ds(c * SEQ + t * 128, 128)],
                            xs[:, t, ts(c, 128)],
                            ident[:, :],
                        )
                xts = sb_xt.tile([128, EC, SEQ], MD, tag="xts")
                nc.any.tensor_copy(
                    xts[:, :, :],
                    xt_ps[:, :].rearrange("p (c s) -> p c s", c=EC),
                )

                # 3. [kT; vT] projection: [Wk|Wv].T @ xT, accumulated over chunks
                kv_ps = ps_qk.tile([128, SEQ], F32, tag="kv")
                for c in range(EC):
                    nc.tensor.matmul(
                        kv_ps[:, :],
                        wkv_sb[:, c, :],
                        xts[:, c, :],
                        start=(c == 0),
                        stop=(c == EC - 1),
                    )
                kv_sb = sb_qk.tile([128, SEQ], MD, tag="kv_sb")
                nc.any.tensor_copy(kv_sb[:, :], kv_ps[:, :])

                # qT projection
                q_ps = ps_vt.tile([HEAD_DIM, SEQ], F32, tag="q")
                for c in range(EC):
                    nc.tensor.matmul(
                        q_ps[:, :],
                        wq_sb[:, c, :],
                        xts[:, c, :],
                        start=(c == 0),
                        stop=(c == EC - 1),
                    )
                qt_sb = sb_qk.tile([HEAD_DIM, SEQ], MD, tag="qt_sb")
                nc.any.tensor_copy(qt_sb[:, :], q_ps[:, :])

                # 4. transpose vT back to natural v (vT lives at partitions
                # 64:128 of kv_sb; ident[64:,64:] is a 64x64 identity there)
                vn_ps = ps_vn.tile([128, 2 * HEAD_DIM], F32, tag="vn")
                for t in range(ST):
                    kv_in = kv_sb[HEAD_DIM:128, ts(t, 128)]
                    if mm_dt == "f32r":
                        kv_in = kv_in.bitcast(F32)
                    nc.tensor.transpose(
                        vn_ps[:, ts(t, HEAD_DIM)],
                        kv_in,
                        ident[HEAD_DIM:128, HEAD_DIM:128],
                    )
                # v padded to 66 cols: col 64 and 65 are ones (fp32r matmul
                # needs an even moving-dim; col 64 yields the softmax
                # denominator, col 65 is a harmless duplicate)
                v_sb = sb_v.tile([128, ST, HEAD_DIM + 2], MD, tag="v_sb")
                nc.any.tensor_copy(
                    v_sb[:, :, 0:HEAD_DIM],
                    vn_ps[:, :].rearrange("p (t d) -> p t d", t=ST),
                )
                nc.any.tensor_copy(
                    v_sb[:, :, HEAD_DIM : HEAD_DIM + 2], ones_f32[:, :, :]
                )

                # 5. scores sT[k, q] per k block
                st_ps = ps_st.tile([128, ST, SEQ], F32, tag="st")
                for t in range(ST):
                    nc.tensor.matmul(
                        st_ps[:, t, :],
                        kv_sb[0:HEAD_DIM, ts(t, 128)],
                        qt_sb[:, :],
                        start=True,
                        stop=True,
                    )

                # 6. p = exp(sT/8); causal mask on diagonal blocks
                pt_sb = sb_p.tile([128, ST, SEQ], MD, tag="pt")
                nc.scalar.activation(
                    pt_sb[:, 0, :],
                    st_ps[:, 0, :],
                    mybir.ActivationFunctionType.Exp,
                    scale=0.125,
                )
                nc.scalar.activation(
                    pt_sb[:, 1, 128:256],
                    st_ps[:, 1, 128:256],
                    mybir.ActivationFunctionType.Exp,
                    scale=0.125,
                )
                nc.vector.tensor_mul(
                    pt_sb[:, 0, 0:128], pt_sb[:, 0, 0:128], tri[:, :]
                )
                nc.vector.tensor_mul(
                    pt_sb[:, 1, 128:256], pt_sb[:, 1, 128:256], tri[:, :]
                )

                # 7. out_unnorm = p.T @ [v|1|1]  (col 64 = softmax denominator)
                AW = HEAD_DIM + 2
                av_ps = ps_av.tile([128, 2 * AW], F32, tag="av")
                o0 = av_ps[:, 0:AW]
                o1 = av_ps[:, AW : 2 * AW]
                nc.tensor.matmul(
                    o0, pt_sb[:, 0, 0:128], v_sb[:, 0, :],
                    start=True, stop=True,
                )
                nc.tensor.matmul(
                    o1, pt_sb[:, 0, 128:256], v_sb[:, 0, :],
                    start=True, stop=False,
                )
                nc.tensor.matmul(
                    o1, pt_sb[:, 1, 128:256], v_sb[:, 1, :],
                    start=False, stop=True,
                )

                # 8. normalize rows and store
                linv = sb_o.tile([128, ST], F32, tag="linv")
                out_sb = sb_o.tile([128, ST, HEAD_DIM], F32, tag="out_sb")
                for t in range(ST):
                    col = t * AW
                    nc.vector.reciprocal(
                        linv[:, t : t + 1],
                        av_ps[:, col + HEAD_DIM : col + HEAD_DIM + 1],
                    )
                    nc.vector.tensor_scalar_mul(
                        out_sb[:, t, :],
                        av_ps[:, ds(col, HEAD_DIM)],
                        linv[:, t : t + 1],
                    )
                nc.scalar.dma_start(out=ov[b], in_=out_sb[:, :, :])

    nc.compile()
    return nc


_NC_CACHE = {}


def _get_nc(nb=NB, mm_dt="f32r"):
    key = (nb, mm_dt)
    if key not in _NC_CACHE:
        _NC_CACHE[key] = _build(nb, mm_dt)
    return _NC_CACHE[key]


def kernel(input_tensor, Wq, Wk, Wv, **run_kwargs):
    x = np.ascontiguousarray(np.asarray(input_tensor, dtype=np.float32))
    wq = np.ascontiguousarray(np.asarray(Wq, dtype=np.float32))
    wk = np.ascontiguousarray(np.asarray(Wk, dtype=np.float32))
    wv = np.ascontiguousarray(np.asarray(Wv, dtype=np.float32))

    nb = x.shape[0] // NCORES
    nc = _get_nc(nb=nb)
    in_maps = [
        {"x": x[i * nb : (i + 1) * nb], "wq": wq, "wk": wk, "wv": wv}
        for i in range(NCORES)
    ]
    res = run_bass_kernel_spmd(nc, in_maps, core_ids=list(range(NCORES)), **run_kwargs)
    outs = np.concatenate([res.results[i]["out"] for i in range(NCORES)], axis=0)
    if run_kwargs.get("trace"):
        kernel.last_results = res
    return outs


# revision 7
# speedup vs baseline: 1.7962x; 1.7962x over previous
"""Causal single-head attention on 8 Trainium2 NeuronCores.

Shapes (hardcoded per problem spec):
  input_tensor [512, 256, 384] f32, Wq/Wk/Wv [384, 64] f32 -> out [512, 256, 64] f32

Sharding: data-parallel on the batch dim, 64 batches per core, weights
replicated.

Per-batch pipeline on each core (S=256 split into two 128-row blocks,
E=384 split into three 128-row chunks):
  1. DMA x_b [256,384] into SBUF with an f32->f16 cast (SWDGE).
  2. PE-transpose the six 128x128 blocks -> xT [384(3 chunks),256] (f16 PSUM,
     exact) then copy to SBUF.
  3. [kT;vT] = [Wk|Wv].T @ xT -> [128,256] (kT at partitions 0:64, vT at
     64:128); qT = Wq.T @ xT -> [64,256].  f16 inputs, f32 accumulation.
  4. PE-transpose vT back to natural v [256,64], append two ones columns
     (col 64 gives the softmax denominator inside the AV matmul).
  5. sT[k,q] = kT_block.T @ qT for both k blocks -> [128,2,256] f32 PSUM.
  6. p = exp(0.125 * sT) on ScalarE (no max subtraction: scores ~ N(0,1), the
     softmax is shift-invariant and exp stays in range), then multiply the two
     diagonal blocks by an upper-triangular 0/1 mask (k<=q).
  7. out_unnorm[q,:] = p_block.T @ [v|1|1], accumulated over the causal k
     blocks only; col 64 = sum_k p = softmax denominator l.
  8. out = out_unnorm[:, :64] * (1/l) in f32, DMA to HBM.

All matmul inputs are fp16 (1 cycle/row on the PE = 4x the fp32 rate, fast
weight loads); every contraction accumulates in f32 PSUM, and the final
normalize runs in f32.
"""

import numpy as np

import concourse.bass as bass
import concourse.mybir as mybir
import concourse.tile as tile
from concourse import bacc
from concourse.bass import ds, ts
from concourse.bass_utils import run_bass_kernel_spmd
from concourse.masks import make_identity, make_upper_triangular

EMBED = 384
HEAD_DIM = 64
SEQ = 256
BATCH = 512
NCORES = 8
NB = BATCH // NCORES  # batches per core

F32 = mybir.dt.float32
F16 = mybir.dt.float16
BF16 = mybir.dt.bfloat16

EC = EMBED // 128  # 3 embed chunks
ST = SEQ // 128    # 2 seq blocks


def _build(nb=NB, mm_dt="f16"):
    """Build the per-core Bass program for nb batches."""
    MD = {"f16": F16, "bf16": BF16}[mm_dt]

    nc = bacc.Bacc("TRN2", target_bir_lowering=False)
    x = nc.dram_tensor("x", [nb, SEQ, EMBED], F32, kind="ExternalInput")
    wq = nc.dram_tensor("wq", [EMBED, HEAD_DIM], F32, kind="ExternalInput")
    wk = nc.dram_tensor("wk", [EMBED, HEAD_DIM], F32, kind="ExternalInput")
    wv = nc.dram_tensor("wv", [EMBED, HEAD_DIM], F32, kind="ExternalInput")
    out = nc.dram_tensor("out", [nb, SEQ, HEAD_DIM], F32, kind="ExternalOutput")

    xv = x[:, :, :].rearrange("b (t p) e -> b p t e", p=128)
    ov = out[:, :, :].rearrange("b (t p) d -> b p t d", p=128)

    with tile.TileContext(nc) as tc:
        with (
            tc.tile_pool(name="const", bufs=1) as cpool,
            tc.tile_pool(name="sb_x", bufs=3) as sb_x,
            tc.tile_pool(name="sb_xt", bufs=3) as sb_xt,
            tc.tile_pool(name="sb_qk", bufs=3) as sb_qk,
            tc.tile_pool(name="sb_v", bufs=3) as sb_v,
            tc.tile_pool(name="sb_p", bufs=3) as sb_p,
            tc.tile_pool(name="sb_o", bufs=3) as sb_o,
            tc.tile_pool(name="ps_xt", bufs=2, space="PSUM") as ps_xt,
            tc.tile_pool(name="ps_kv", bufs=2, space="PSUM") as ps_kv,
            tc.tile_pool(name="ps_q", bufs=1, space="PSUM") as ps_q,
            tc.tile_pool(name="ps_vn", bufs=1, space="PSUM") as ps_vn,
            tc.tile_pool(name="ps_st", bufs=1, space="PSUM") as ps_st,
            tc.tile_pool(name="ps_av", bufs=1, space="PSUM") as ps_av,
        ):
            ident = cpool.tile([128, 128], MD)
            make_identity(nc, ident)
            # tri[k, q] = 1.0 where k <= q else 0.0
            tri = cpool.tile([128, 128], MD)
            make_upper_triangular(nc, tri, val=1.0, diag=True)

            # [Wk|Wv] packed: projection puts kT at partitions 0:64 (base 0,
            # as the scores matmul needs) and vT at 64:128 (only feeds the
            # PE transpose, which works at base 64 with ident[64:,64:]).
            # gpsimd DMA casts f32 -> f16 on the fly.
            wkv_sb = cpool.tile([128, EC, 128], MD)
            nc.gpsimd.dma_start(
                out=wkv_sb[:, :, 0:HEAD_DIM],
                in_=wk[:, :].rearrange("(c p) d -> p c d", p=128),
            )
            nc.gpsimd.dma_start(
                out=wkv_sb[:, :, HEAD_DIM:128],
                in_=wv[:, :].rearrange("(c p) d -> p c d", p=128),
            )
            wq_sb = cpool.tile([128, EC, HEAD_DIM], MD)
            nc.gpsimd.dma_start(
                out=wq_sb[:, :, :],
                in_=wq[:, :].rearrange("(c p) d -> p c d", p=128),
            )

            for b in range(nb):
                # 1. load x_b with f32 -> f16 cast
                xs = sb_x.tile([128, ST, EMBED], MD, tag="xs")
                nc.gpsimd.dma_start(out=xs[:, :, :], in_=xv[b])

                # 2. transpose x -> xT  (blocks at col c*256 + t*128)
                xt_ps = ps_xt.tile([128, EC * SEQ], MD, tag="xt")
                for t in range(ST):
                    for c in range(EC):
                        nc.tensor.transpose(
                            xt_ps[:, ds(c * SEQ + t * 128, 128)],
                            xs[:, t, ts(c, 128)],
                            ident[:, :],
                        )
                xts = sb_xt.tile([128, EC, SEQ], MD, tag="xts")
                nc.any.tensor_copy(
                    xts[:, :, :],
                    xt_ps[:, :].rearrange("p (c s) -> p c s", c=EC),
                )

                # 3. [kT; vT] projection: [Wk|Wv].T @ xT, accumulated over chunks
                kv_ps = ps_kv.tile([128, SEQ], F32, tag="kv")
                for c in range(EC):
                    nc.tensor.matmul(
                        kv_ps[:, :],
                        wkv_sb[:, c, :],
                        xts[:, c, :],
                        start=(c == 0),
                        stop=(c == EC - 1),
                    )
                kv_sb = sb_qk.tile([128, SEQ], MD, tag="kv_sb")
                nc.any.tensor_copy(kv_sb[:, :], kv_ps[:, :])

                # qT projection
                q_ps = ps_q.tile([HEAD_DIM, SEQ], F32, tag="q")
                for c in range(EC):
                    nc.tensor.matmul(
                        q_ps[:, :],
                        wq_sb[:, c, :],
                        xts[:, c, :],
                        start=(c == 0),
                        stop=(c == EC - 1),
                    )
                qt_sb = sb_qk.tile([HEAD_DIM, SEQ], MD, tag="qt_sb")
                nc.any.tensor_copy(qt_sb[:, :], q_ps[:, :])

                # 4. transpose vT back to natural v (vT lives at partitions
                # 64:128 of kv_sb; ident[64:,64:] is a 64x64 identity there)
                vn_ps = ps_vn.tile([128, 2 * HEAD_DIM], MD, tag="vn")
                for t in range(ST):
                    nc.tensor.transpose(
                        vn_ps[:, ts(t, HEAD_DIM)],
                        kv_sb[HEAD_DIM:128, ts(t, 128)],
                        ident[HEAD_DIM:128, HEAD_DIM:128],
                    )
                # v padded to 66 cols: cols 64/65 are ones (col 64 yields the
                # softmax denominator, col 65 is a harmless duplicate that
                # keeps the moving dim even)
                v_sb = sb_v.tile([128, ST, HEAD_DIM + 2], MD, tag="v_sb")
                nc.any.tensor_copy(
                    v_sb[:, :, 0:HEAD_DIM],
                    vn_ps[:, :].rearrange("p (t d) -> p t d", t=ST),
                )
                nc.vector.memset(v_sb[:, :, HEAD_DIM : HEAD_DIM + 2], 1.0)

                # 5. scores sT[k, q] per k block
                st_ps = ps_st.tile([128, ST, SEQ], F32, tag="st")
                for t in range(ST):
                    nc.tensor.matmul(
                        st_ps[:, t, :],
                        kv_sb[0:HEAD_DIM, ts(t, 128)],
                        qt_sb[:, :],
                        start=True,
                        stop=True,
                    )

                # 6. p = exp(sT/8); causal mask on diagonal blocks
                pt_sb = sb_p.tile([128, ST, SEQ], MD, tag="pt")
                nc.scalar.activation(
                    pt_sb[:, 0, :],
                    st_ps[:, 0, :],
                    mybir.ActivationFunctionType.Exp,
                    scale=0.125,
                )
                nc.scalar.activation(
                    pt_sb[:, 1, 128:256],
                    st_ps[:, 1, 128:256],
                    mybir.ActivationFunctionType.Exp,
                    scale=0.125,
                )
                nc.vector.tensor_mul(
                    pt_sb[:, 0, 0:128], pt_sb[:, 0, 0:128], tri[:, :]
                )
                nc.vector.tensor_mul(
                    pt_sb[:, 1, 128:256], pt_sb[:, 1, 128:256], tri[:, :]
                )

                # 7. out_unnorm = p.T @ [v|1|1]  (col 64 = softmax denominator)
                AW = HEAD_DIM + 2
                av_ps = ps_av.tile([128, 2 * AW], F32, tag="av")
                o0 = av_ps[:, 0:AW]
                o1 = av_ps[:, AW : 2 * AW]
                nc.tensor.matmul(
                    o0, pt_sb[:, 0, 0:128], v_sb[:, 0, :],
                    start=True, stop=True,
                )
                nc.tensor.matmul(
                    o1, pt_sb[:, 0, 128:256], v_sb[:, 0, :],
                    start=True, stop=False,
                )
                nc.tensor.matmul(
                    o1, pt_sb[:, 1, 128:256], v_sb[:, 1, :],
                    start=False, stop=True,
                )

                # 8. normalize rows (f32) and store
                linv = sb_o.tile([128, ST], F32, tag="linv")
                out_sb = sb_o.tile([128, ST, HEAD_DIM], F32, tag="out_sb")
                for t in range(ST):
                    col = t * AW
                    nc.vector.reciprocal(
                        linv[:, t : t + 1],
                        av_ps[:, col + HEAD_DIM : col + HEAD_DIM + 1],
                    )
                    nc.vector.tensor_scalar_mul(
                        out_sb[:, t, :],
                        av_ps[:, ds(col, HEAD_DIM)],
                        linv[:, t : t + 1],
                    )
                nc.scalar.dma_start(out=ov[b], in_=out_sb[:, :, :])

    nc.compile()
    return nc


_NC_CACHE = {}


def _get_nc(nb=NB, mm_dt="f16"):
    key = (nb, mm_dt)
    if key not in _NC_CACHE:
        _NC_CACHE[key] = _build(nb, mm_dt)
    return _NC_CACHE[key]


def kernel(input_tensor, Wq, Wk, Wv, **run_kwargs):
    x = np.ascontiguousarray(np.asarray(input_tensor, dtype=np.float32))
    wq = np.ascontiguousarray(np.asarray(Wq, dtype=np.float32))
    wk = np.ascontiguousarray(np.asarray(Wk, dtype=np.float32))
    wv = np.ascontiguousarray(np.asarray(Wv, dtype=np.float32))

    nb = x.shape[0] // NCORES
    nc = _get_nc(nb=nb)
    in_maps = [
        {"x": x[i * nb : (i + 1) * nb], "wq": wq, "wk": wk, "wv": wv}
        for i in range(NCORES)
    ]
    res = run_bass_kernel_spmd(nc, in_maps, core_ids=list(range(NCORES)), **run_kwargs)
    outs = np.concatenate([res.results[i]["out"] for i in range(NCORES)], axis=0)
    if run_kwargs.get("trace"):
        kernel.last_results = res
    return outs
